# revision 30
# baseline (speedup 1.0000x reference)
"""Multi-head causal attention with RoPE on 8 Trainium2 cores.

Sharding: batch (2) x head-groups (4 heads each) -> 8 shards, one per core.
All matmuls run in bf16 (streams 1 col/cycle at any width, vs fp32r's 4x
penalty below 256 cols), accumulating in fp32 PSUM. Host pre-casts inputs
to bf16 and pre-packs per-core layouts.

Per-core dataflow (batch b, 4 heads):
  stage A: qT/kT feature-major [(4x64), T] = w.T @ x.T  (+bias, RoPE via
    row-swap DMA + 3 bf16 DVE passes); v token-major [T, 4x(64+ones-col)].
    PSUM evictions on the Pool engine (tensor_scalar add / copy).
  stage B: per (query-chunk, head): S^T[j,i] = k-chunk.T @ q (causal blocks,
    full-width key-block pairs share one 2-bank PSUM tile so exp handles
    [128,2,512] in one Act instruction), P = exp(S/8) in bf16, diagonal
    masked by a bf16 DVE multiply, PV accumulated feature-major with a
    ones-column giving Z in PSUM row 64. Normalize: 1/Z via DVE
    reciprocal_approx_fast, partition-broadcast via a 1-row ones matmul on
    the PE, one DVE multiply -> oTn bf16.
  stage C: outT = w_out_rows.T @ oTn per 512-col chunk, Pool eviction to
    bf16 staging rows, one output DMA per 128-row stripe.
Host sums the 4 head-group partials per batch and adds biases.
"""
import numpy as np

B, T, E, H = 2, 2048, 1024, 16
D = 64
HPC = 4           # heads per core
CG = HPC * D      # 256 channels per shard
NE = E // 128     # 8 contraction chunks
NJ = T // 128     # 16 key tiles
NCH = T // 512    # 4 query chunks
ROPE_BASE = 10000.0

_CACHE = {}


def _host_constants():
    import ml_dtypes
    t = np.arange(T, dtype=np.float32)
    inv_freq = (1.0 / (ROPE_BASE ** (np.arange(0, D, 2, dtype=np.float32) / D))).astype(np.float32)
    freqs = t[:, None] * inv_freq[None, :]          # [T, 32]
    fcos = np.cos(freqs).T.astype(np.float32)       # [32, T]
    fsin = np.sin(freqs).T.astype(np.float32)
    cosT = np.vstack([fcos, fcos])                  # [64, T]
    sinnT = np.vstack([-fsin, fsin])                # [64, T] sign-folded for rotate_half
    cos2 = np.ascontiguousarray(np.vstack([cosT, cosT])).astype(ml_dtypes.bfloat16)
    sinn2 = np.ascontiguousarray(np.vstack([sinnT, sinnT])).astype(ml_dtypes.bfloat16)
    mask = np.triu(np.ones((128, 128), dtype=np.float32)).astype(ml_dtypes.bfloat16)
    return cos2, sinn2, mask


def _build(repeat=1):
    import concourse.bacc as bacc
    import concourse.mybir as mybir
    import concourse.tile as tile

    F32 = mybir.dt.float32
    F32R = mybir.dt.float32r
    BF16 = mybir.dt.bfloat16
    AF = mybir.ActivationFunctionType

    nc = bacc.Bacc("TRN2", target_bir_lowering=False, debug=False, enable_asserts=True)

    x8 = nc.dram_tensor("x8", [E, T], BF16, kind="ExternalInput").ap()
    wqkv = nc.dram_tensor("wqkv", [E, 3 * CG], BF16, kind="ExternalInput").ap()
    wo = nc.dram_tensor("wo", [CG, E], BF16, kind="ExternalInput").ap()
    cos2 = nc.dram_tensor("cos2", [128, T], BF16, kind="ExternalInput").ap()
    sinn2 = nc.dram_tensor("sinn2", [128, T], BF16, kind="ExternalInput").ap()
    mask = nc.dram_tensor("mask", [128, 128], BF16, kind="ExternalInput").ap()
    bqk = nc.dram_tensor("bqk", [128, 4], F32, kind="ExternalInput").ap()
    ones1 = nc.dram_tensor("ones1", [128, HPC], BF16, kind="ExternalInput").ap()
    outT = nc.dram_tensor("outT", [E, T], BF16, kind="ExternalOutput").ap()

    with tile.TileContext(nc) as tc:
        with tc.tile_pool(name="persist", bufs=1) as pp:
            q_t = [pp.tile([128, T], BF16, tag=f"q{i}", name=f"q{i}") for i in range(2)]
            k_t = [pp.tile([128, T], BF16, tag=f"k{i}", name=f"k{i}") for i in range(2)]
            v_t = [pp.tile([128, HPC, 65], BF16, tag=f"v{j}", name=f"v{j}") for j in range(NJ)]
            oTn = [pp.tile([128, T], BF16, tag=f"o{i}", name=f"o{i}") for i in range(2)]
            ob = [pp.tile([128, T], BF16, tag=f"ob{i}", name=f"ob{i}") for i in range(NE)]
            wo_sb = [pp.tile([128, E], BF16, tag=f"wo{i}", name=f"wosb{i}") for i in range(2)]
            cos_sb = pp.tile([128, T], BF16, tag="cos")
            sinn_sb = pp.tile([128, T], BF16, tag="sinn")
            mask_sb = pp.tile([128, 128], BF16, tag="mask")
            bqk_sb = pp.tile([128, 4], F32, tag="bqk")

            for _rep in range(repeat):
                # ---- stage A: QKV projection + RoPE ----
                with tc.tile_pool(name="qkv", bufs=1) as qp, \
                     tc.tile_pool(name="rope", bufs=2) as rp, \
                     tc.tile_pool(name="qkv_psum", bufs=8, space="PSUM") as qpp:
                    xts = [qp.tile([128, T], BF16, tag=f"xt{e}", name=f"xt{e}") for e in range(NE)]
                    w_sb = [qp.tile([128, 3 * CG], BF16, tag=f"w{e}", name=f"wsb{e}") for e in range(NE)]
                    # first-needed data first: weights + x quarter 0 per e-chunk
                    for e in range(NE):
                        sl = slice(128 * e, 128 * (e + 1))
                        nc.sync.dma_start(out=w_sb[e], in_=wqkv[sl, :])
                        nc.sync.dma_start(out=xts[e][:, 0:512], in_=x8[sl, 0:512])
                    for e in range(NE):
                        sl = slice(128 * e, 128 * (e + 1))
                        nc.sync.dma_start(out=xts[e][:, 512:T], in_=x8[sl, 512:T])
                    nc.gpsimd.dma_start(out=bqk_sb, in_=bqk)
                    nc.gpsimd.dma_start(out=cos_sb, in_=cos2)
                    nc.gpsimd.dma_start(out=sinn_sb, in_=sinn2)
                    nc.gpsimd.dma_start(out=mask_sb, in_=mask)
                    for i in range(2):
                        nc.gpsimd.dma_start(out=wo_sb[i], in_=wo[128 * i:128 * (i + 1), :])
                    for j in range(NJ):
                        nc.gpsimd.dma_start(
                            out=v_t[j][:, :, 64:65],
                            in_=ones1.rearrange("p (h o) -> p h o", o=1))

                    # q, k feature-major + v token-major, per x-quarter
                    for tch in range(NCH):
                        tsl = slice(512 * tch, 512 * (tch + 1))
                        for qk, dst in ((0, q_t), (1, k_t)):
                            for ct in range(2):
                                ps = qpp.tile([128, 512], F32, tag="qkvp", name="psqk")
                                for e in range(NE):
                                    nc.tensor.matmul(
                                        ps,
                                        lhsT=w_sb[e][:, CG * qk + 128 * ct:CG * qk + 128 * (ct + 1)],
                                        rhs=xts[e][:, tsl],
                                        start=(e == 0), stop=(e == NE - 1),
                                    )
                                nc.scalar.activation(
                                    out=dst[ct][:, tsl], in_=ps, func=AF.Identity,
                                    bias=bqk_sb[:, 2 * qk + ct:2 * qk + ct + 1],
                                )
                        for j in range(4 * tch, 4 * tch + 4):
                            ps = qpp.tile([128, 512], F32, tag="qkvp", name="psv")
                            for e in range(NE):
                                nc.tensor.matmul(
                                    ps[:, 0:CG],
                                    lhsT=xts[e][:, 128 * j:128 * (j + 1)],
                                    rhs=w_sb[e][:, 2 * CG:3 * CG],
                                    start=(e == 0), stop=(e == NE - 1),
                                )
                            nc.vector.tensor_copy(
                                out=v_t[j][:, :, 0:64],
                                in_=ps[:, 0:CG].rearrange("p (h d) -> p h d", h=HPC),
                            )
                        if tch == 0:
                            # RoPE on the first 512 cols so attention ch=0 can start
                            for t_ in (q_t[0], k_t[0], q_t[1], k_t[1]):
                                swq = rp.tile([128, 512], BF16, tag="swq", name="swq", bufs=4)
                                for hh in (0, 64):
                                    nc.sync.dma_start(out=swq[hh:hh + 32, :], in_=t_[hh + 32:hh + 64, 0:512])
                                    nc.sync.dma_start(out=swq[hh + 32:hh + 64, :], in_=t_[hh:hh + 32, 0:512])
                                nc.vector.tensor_mul(out=swq, in0=swq, in1=sinn_sb[:, 0:512])
                                nc.vector.tensor_mul(out=t_[:, 0:512], in0=t_[:, 0:512], in1=cos_sb[:, 0:512])
                                nc.vector.tensor_add(out=t_[:, 0:512], in0=t_[:, 0:512], in1=swq)

                    # RoPE on cols 512:T, one wide pass per tile
                    for t_ in (q_t[0], k_t[0], q_t[1], k_t[1]):
                        sw = rp.tile([128, T - 512], BF16, tag="swr", name="swr", bufs=2)
                        for hh in (0, 64):
                            nc.sync.dma_start(out=sw[hh:hh + 32, :], in_=t_[hh + 32:hh + 64, 512:T])
                            nc.sync.dma_start(out=sw[hh + 32:hh + 64, :], in_=t_[hh:hh + 32, 512:T])
                        cs = slice(512, T)
                        nc.vector.tensor_mul(out=sw, in0=sw, in1=sinn_sb[:, cs])
                        nc.vector.tensor_mul(out=t_[:, cs], in0=t_[:, cs], in1=cos_sb[:, cs])
                        nc.vector.tensor_add(out=t_[:, cs], in0=t_[:, cs], in1=sw)

                # ---- stage B: causal attention, software-pipelined ----
                # Work units stream through (S matmul -> exp -> [mask] -> PV);
                # PV emission lags S/exp by PIPE units so the PE never waits
                # on the Act engine's exp. The 1/Z broadcast shares the PV
                # psum bank (partitions 64:128) so two PV banks fit.
                with tc.tile_pool(name="attn", bufs=8) as ap_, \
                     tc.tile_pool(name="norm", bufs=4) as np_, \
                     tc.tile_pool(name="s_psum", bufs=3, space="PSUM") as sp, \
                     tc.tile_pool(name="pv_psum", bufs=2, space="PSUM") as pvp:
                    PIPE = 2
                    pending = []  # deferred emission closures (PV matmuls, normalize)

                    def drain(n):
                        while len(pending) > n:
                            pending.pop(0)()

                    def mk_pv(pv, lhsT, rhs_p, st, sp_, c0):
                        def emit():
                            nc.tensor.matmul(pv[0:65, c0:512], lhsT=lhsT, rhs=rhs_p,
                                             start=st, stop=sp_, skip_group_check=True)
                        return emit

                    def mk_norm(pv, ct, poff, i0):
                        def emit():
                            rz = np_.tile([1, 512], F32, tag="rz")
                            with nc.allow_low_precision(reason="softmax denominators"):
                                nc.vector.reciprocal(out=rz, in_=pv[64:65, :])
                            bc = np_.tile([64, 512], F32, tag="bc")
                            nc.gpsimd.partition_broadcast(bc, rz)
                            nc.vector.tensor_mul(
                                out=oTn[ct][poff:poff + 64, i0:i0 + 512],
                                in0=pv[0:64, :], in1=bc,
                            )
                        return emit

                    for ch in (0, 3, 2, 1):
                        i0 = 512 * ch
                        for h in range(HPC):
                            ct, poff = h // 2, 64 * (h % 2)
                            pv = pvp.tile([128, 512], F32, tag="pv")
                            qs = q_t[ct][poff:poff + 64, i0:i0 + 512]
                            nmm = 4 * ch + 4
                            # paired full-width key blocks: (0,1), (2,3), ...
                            for m in range(2 * ch):
                                ja, jb = 2 * m, 2 * m + 1
                                s2 = sp.tile([128, 2, 512], F32, tag="s2")
                                for g, j in ((0, ja), (1, jb)):
                                    nc.tensor.matmul(
                                        s2[:, g, :],
                                        lhsT=k_t[ct][poff:poff + 64, 128 * j:128 * (j + 1)],
                                        rhs=qs, start=True, stop=True,
                                    )
                                p2 = ap_.tile([128, 2, 512], BF16, tag="p2")
                                nc.scalar.activation(out=p2, in_=s2, func=AF.Exp, scale=0.125)
                                for g, j in ((0, ja), (1, jb)):
                                    pending.append(mk_pv(pv, v_t[j][:, h, :], p2[:, g, :],
                                                         j == 0, j == nmm - 1, 0))
                                drain(PIPE)
                            # leftover full block + 3 tail blocks (w 384/256/128)
                            for j in range(4 * ch, 4 * ch + 4):
                                j0 = 128 * j
                                w = i0 + 512 - j0
                                c0 = j0 - i0
                                s2 = sp.tile([128, 2, 512], F32, tag="s2")
                                nc.tensor.matmul(
                                    s2[:, 0, 0:w],
                                    lhsT=k_t[ct][poff:poff + 64, j0:j0 + 128],
                                    rhs=q_t[ct][poff:poff + 64, j0:i0 + 512],
                                    start=True, stop=True,
                                )
                                p1 = ap_.tile([128, 512], BF16, tag="p1")
                                nc.scalar.activation(out=p1[:, 0:w], in_=s2[:, 0, 0:w],
                                                     func=AF.Exp, scale=0.125)
                                nc.vector.tensor_mul(out=p1[:, 0:128],
                                                     in0=p1[:, 0:128], in1=mask_sb)
                                pending.append(mk_pv(pv, v_t[j][:, h, :], p1[:, 0:w],
                                                     j == 0, j == nmm - 1, c0))
                                drain(PIPE)
                            # normalize lags through the pipeline like PV units
                            pending.append(mk_norm(pv, ct, poff, i0))
                    drain(0)

                # ---- stage C: output projection ----
                with tc.tile_pool(name="out_psum", bufs=6, space="PSUM") as opp:
                    dma_eng = [nc.sync, nc.scalar, nc.sync, nc.gpsimd]
                    for tch in (0, 3, 2, 1):
                        tsl = slice(512 * tch, 512 * (tch + 1))
                        for et in range(NE):
                            ps = opp.tile([128, 512], F32, tag="op")
                            for cc in range(2):
                                nc.tensor.matmul(
                                    ps,
                                    lhsT=wo_sb[cc][:, 128 * et:128 * (et + 1)],
                                    rhs=oTn[cc][:, tsl],
                                    start=(cc == 0), stop=(cc == 1),
                                )
                            if et % 2 == 0:
                                nc.scalar.copy(out=ob[et][:, tsl], in_=ps)
                            else:
                                nc.vector.tensor_copy(out=ob[et][:, tsl], in_=ps)
                            dma_eng[et % 4].dma_start(
                                out=outT[128 * et:128 * (et + 1), tsl],
                                in_=ob[et][:, tsl])

    nc.compile()
    return nc


def get_nc(repeat=1):
    key = f"nc{repeat}"
    if key not in _CACHE:
        _CACHE[key] = _build(repeat)
    return _CACHE[key]


def make_in_maps(x, w_qkv, b_qkv):
    import ml_dtypes
    cos2, sinn2, mask = _host_constants()
    x = np.asarray(x, dtype=np.float32)
    w_qkv = np.asarray(w_qkv, dtype=np.float32)
    b_qkv = np.asarray(b_qkv, dtype=np.float32)
    xT8 = [np.ascontiguousarray(x[b].T).astype(ml_dtypes.bfloat16) for b in range(B)]
    in_maps = []
    for c in range(8):
        b, hg = divmod(c, 4)
        sl = slice(CG * hg, CG * (hg + 1))
        wq = w_qkv[:, 0 * E:1 * E][:, sl]
        wk = w_qkv[:, 1 * E:2 * E][:, sl]
        wv = w_qkv[:, 2 * E:3 * E][:, sl]
        bq = b_qkv[0 * E:1 * E][sl]
        bk = b_qkv[1 * E:2 * E][sl]
        # bqk layout [128, 4]: (bq ct0, bq ct1, bk ct0, bk ct1)
        bqk = np.stack([bq[0:128], bq[128:256], bk[0:128], bk[128:256]],
                       axis=1).astype(np.float32)
        in_maps.append({
            "x8": xT8[b],
            "wqkv": np.ascontiguousarray(
                np.concatenate([wq, wk, wv], axis=1)).astype(ml_dtypes.bfloat16),
            "wo": None,  # filled by caller (needs w_out)
            "cos2": cos2, "sinn2": sinn2, "mask": mask,
            "bqk": np.ascontiguousarray(bqk),
            "ones1": np.ones((128, HPC), dtype=ml_dtypes.bfloat16),
        })
    return in_maps


def fill_wo(in_maps, w_out):
    import ml_dtypes
    w_out = np.asarray(w_out, dtype=np.float32)
    for c in range(8):
        hg = c % 4
        in_maps[c]["wo"] = np.ascontiguousarray(
            w_out[CG * hg:CG * (hg + 1), :]).astype(ml_dtypes.bfloat16)
    return in_maps


def kernel(x, w_qkv, b_qkv, w_out, b_out, _res_out=None):
    from concourse.bass_utils import run_bass_kernel_spmd

    x = np.asarray(x, dtype=np.float32)
    w_qkv = np.asarray(w_qkv, dtype=np.float32)
    b_qkv = np.asarray(b_qkv, dtype=np.float32)
    w_out = np.asarray(w_out, dtype=np.float32)
    b_out = np.asarray(b_out, dtype=np.float32)

    nc = get_nc()
    in_maps = fill_wo(make_in_maps(x, w_qkv, b_qkv), w_out)

    res = run_bass_kernel_spmd(nc, in_maps, list(range(8)))
    if _res_out is not None:
        _res_out.append(res)

    out = np.empty((B, T, E), np.float32)
    for b in range(B):
        acc = res.results[4 * b + 0]["outT"].astype(np.float32)
        for g in range(1, 4):
            acc += res.results[4 * b + g]["outT"].astype(np.float32)
        out[b] = acc.T
    bias = b_qkv[2 * E:3 * E].astype(np.float64) @ w_out.astype(np.float64) + b_out
    out += bias.astype(np.float32)[None, None, :]
    return out


# revision 31
# speedup vs baseline: 1.1654x; 1.1654x over previous
"""Multi-head causal attention with RoPE on 8 Trainium2 cores.

Sharding: batch (2) x head-groups (4 heads each) -> 8 shards, one per core.
All matmuls run in bf16 (streams 1 col/cycle at any width, vs fp32r's 4x
penalty below 256 cols), accumulating in fp32 PSUM. Host pre-casts inputs
to bf16 and pre-packs per-core layouts.

Per-core dataflow (batch b, 4 heads):
  stage A: qT/kT feature-major [(4x64), T] = w.T @ x.T  (+bias, RoPE via
    row-swap DMA + 3 bf16 DVE passes); v token-major [T, 4x(64+ones-col)].
    PSUM evictions on the Pool engine (tensor_scalar add / copy).
  stage B: per (query-chunk, head): S^T[j,i] = k-chunk.T @ q (causal blocks,
    full-width key-block pairs share one 2-bank PSUM tile so exp handles
    [128,2,512] in one Act instruction), P = exp(S/8) in bf16, diagonal
    masked by a bf16 DVE multiply, PV accumulated feature-major with a
    ones-column giving Z in PSUM row 64. Normalize: 1/Z via DVE
    reciprocal_approx_fast, partition-broadcast via a 1-row ones matmul on
    the PE, one DVE multiply -> oTn bf16.
  stage C: outT = w_out_rows.T @ oTn per 512-col chunk, Pool eviction to
    bf16 staging rows, one output DMA per 128-row stripe.
Host sums the 4 head-group partials per batch and adds biases.
"""
import numpy as np

B, T, E, H = 2, 2048, 1024, 16
D = 64
HPC = 4           # heads per core
CG = HPC * D      # 256 channels per shard
NE = E // 128     # 8 contraction chunks
NJ = T // 128     # 16 key tiles
NCH = T // 512    # 4 query chunks
ROPE_BASE = 10000.0

_CACHE = {}


def _host_constants():
    import ml_dtypes
    t = np.arange(T, dtype=np.float32)
    inv_freq = (1.0 / (ROPE_BASE ** (np.arange(0, D, 2, dtype=np.float32) / D))).astype(np.float32)
    freqs = t[:, None] * inv_freq[None, :]          # [T, 32]
    fcos = np.cos(freqs).T.astype(np.float32)       # [32, T]
    fsin = np.sin(freqs).T.astype(np.float32)
    cosT = np.vstack([fcos, fcos])                  # [64, T]
    sinnT = np.vstack([-fsin, fsin])                # [64, T] sign-folded for rotate_half
    cos2 = np.ascontiguousarray(np.vstack([cosT, cosT])).astype(ml_dtypes.bfloat16)
    sinn2 = np.ascontiguousarray(np.vstack([sinnT, sinnT])).astype(ml_dtypes.bfloat16)
    mask = np.triu(np.ones((128, 128), dtype=np.float32)).astype(ml_dtypes.bfloat16)
    return cos2, sinn2, mask


def _build(repeat=1):
    import concourse.bacc as bacc
    import concourse.mybir as mybir
    import concourse.tile as tile

    F32 = mybir.dt.float32
    F32R = mybir.dt.float32r
    BF16 = mybir.dt.bfloat16
    AF = mybir.ActivationFunctionType

    nc = bacc.Bacc("TRN2", target_bir_lowering=False, debug=False, enable_asserts=True)

    x8 = nc.dram_tensor("x8", [E, T], BF16, kind="ExternalInput").ap()
    wqkv = nc.dram_tensor("wqkv", [E, 3 * CG], BF16, kind="ExternalInput").ap()
    wo = nc.dram_tensor("wo", [CG, E], BF16, kind="ExternalInput").ap()
    cos2 = nc.dram_tensor("cos2", [128, T], BF16, kind="ExternalInput").ap()
    sinn2 = nc.dram_tensor("sinn2", [128, T], BF16, kind="ExternalInput").ap()
    mask = nc.dram_tensor("mask", [128, 128], BF16, kind="ExternalInput").ap()
    bqk = nc.dram_tensor("bqk", [128, 4], F32, kind="ExternalInput").ap()
    ones1 = nc.dram_tensor("ones1", [128, HPC], BF16, kind="ExternalInput").ap()
    outT = nc.dram_tensor("outT", [E, T], BF16, kind="ExternalOutput").ap()

    with tile.TileContext(nc) as tc:
        with tc.tile_pool(name="persist", bufs=1) as pp:
            q_t = [pp.tile([128, T], BF16, tag=f"q{i}", name=f"q{i}") for i in range(2)]
            k_t = [pp.tile([128, T], BF16, tag=f"k{i}", name=f"k{i}") for i in range(2)]
            v_t = [pp.tile([128, HPC, 65], BF16, tag=f"v{j}", name=f"v{j}") for j in range(NJ)]
            oTn = [pp.tile([128, T], BF16, tag=f"o{i}", name=f"o{i}") for i in range(2)]
            ob = [pp.tile([128, T], BF16, tag=f"ob{i}", name=f"ob{i}") for i in range(NE)]
            wo_sb = [pp.tile([128, E], BF16, tag=f"wo{i}", name=f"wosb{i}") for i in range(2)]
            cos_sb = pp.tile([128, T], BF16, tag="cos")
            sinn_sb = pp.tile([128, T], BF16, tag="sinn")
            mask_sb = pp.tile([128, 128], BF16, tag="mask")
            bqk_sb = pp.tile([128, 4], F32, tag="bqk")

            for _rep in range(repeat):
                # ---- stage A: QKV projection + RoPE ----
                with tc.tile_pool(name="qkv", bufs=1) as qp, \
                     tc.tile_pool(name="rope", bufs=2) as rp, \
                     tc.tile_pool(name="qkv_psum", bufs=8, space="PSUM") as qpp:
                    xts = [qp.tile([128, T], BF16, tag=f"xt{e}", name=f"xt{e}") for e in range(NE)]
                    w_sb = [qp.tile([128, 3 * CG], BF16, tag=f"w{e}", name=f"wsb{e}") for e in range(NE)]
                    # first-needed data first: weights + x quarter 0 per e-chunk
                    for e in range(NE):
                        sl = slice(128 * e, 128 * (e + 1))
                        nc.sync.dma_start(out=w_sb[e], in_=wqkv[sl, :])
                        nc.sync.dma_start(out=xts[e][:, 0:512], in_=x8[sl, 0:512])
                    for e in range(NE):
                        sl = slice(128 * e, 128 * (e + 1))
                        nc.sync.dma_start(out=xts[e][:, 512:T], in_=x8[sl, 512:T])
                    nc.gpsimd.dma_start(out=bqk_sb, in_=bqk)
                    nc.gpsimd.dma_start(out=cos_sb, in_=cos2)
                    nc.gpsimd.dma_start(out=sinn_sb, in_=sinn2)
                    nc.gpsimd.dma_start(out=mask_sb, in_=mask)
                    for i in range(2):
                        nc.gpsimd.dma_start(out=wo_sb[i], in_=wo[128 * i:128 * (i + 1), :])
                    for j in range(NJ):
                        nc.gpsimd.dma_start(
                            out=v_t[j][:, :, 64:65],
                            in_=ones1.rearrange("p (h o) -> p h o", o=1))

                    # q, k feature-major + v token-major, per x-quarter
                    for tch in range(NCH):
                        tsl = slice(512 * tch, 512 * (tch + 1))
                        for qk, dst in ((0, q_t), (1, k_t)):
                            for ct in range(2):
                                ps = qpp.tile([128, 512], F32, tag="qkvp", name="psqk")
                                for e in range(NE):
                                    nc.tensor.matmul(
                                        ps,
                                        lhsT=w_sb[e][:, CG * qk + 128 * ct:CG * qk + 128 * (ct + 1)],
                                        rhs=xts[e][:, tsl],
                                        start=(e == 0), stop=(e == NE - 1),
                                    )
                                nc.scalar.activation(
                                    out=dst[ct][:, tsl], in_=ps, func=AF.Identity,
                                    bias=bqk_sb[:, 2 * qk + ct:2 * qk + ct + 1],
                                )
                        for j in range(4 * tch, 4 * tch + 4):
                            ps = qpp.tile([128, 512], F32, tag="qkvp", name="psv")
                            for e in range(NE):
                                nc.tensor.matmul(
                                    ps[:, 0:CG],
                                    lhsT=xts[e][:, 128 * j:128 * (j + 1)],
                                    rhs=w_sb[e][:, 2 * CG:3 * CG],
                                    start=(e == 0), stop=(e == NE - 1),
                                )
                            nc.vector.tensor_copy(
                                out=v_t[j][:, :, 0:64],
                                in_=ps[:, 0:CG].rearrange("p (h d) -> p h d", h=HPC),
                            )
                        if tch == 0:
                            # RoPE on the first 512 cols so attention ch=0 can start
                            for t_ in (q_t[0], k_t[0], q_t[1], k_t[1]):
                                swq = rp.tile([128, 512], BF16, tag="swq", name="swq", bufs=4)
                                for hh in (0, 64):
                                    nc.sync.dma_start(out=swq[hh:hh + 32, :], in_=t_[hh + 32:hh + 64, 0:512])
                                    nc.sync.dma_start(out=swq[hh + 32:hh + 64, :], in_=t_[hh:hh + 32, 0:512])
                                nc.vector.tensor_mul(out=swq, in0=swq, in1=sinn_sb[:, 0:512])
                                nc.vector.tensor_mul(out=t_[:, 0:512], in0=t_[:, 0:512], in1=cos_sb[:, 0:512])
                                nc.vector.tensor_add(out=t_[:, 0:512], in0=t_[:, 0:512], in1=swq)

                    # RoPE on cols 512:T, one wide pass per tile
                    for t_ in (q_t[0], k_t[0], q_t[1], k_t[1]):
                        sw = rp.tile([128, T - 512], BF16, tag="swr", name="swr", bufs=2)
                        for hh in (0, 64):
                            nc.sync.dma_start(out=sw[hh:hh + 32, :], in_=t_[hh + 32:hh + 64, 512:T])
                            nc.sync.dma_start(out=sw[hh + 32:hh + 64, :], in_=t_[hh:hh + 32, 512:T])
                        cs = slice(512, T)
                        nc.vector.tensor_mul(out=sw, in0=sw, in1=sinn_sb[:, cs])
                        nc.vector.tensor_mul(out=t_[:, cs], in0=t_[:, cs], in1=cos_sb[:, cs])
                        nc.vector.tensor_add(out=t_[:, cs], in0=t_[:, cs], in1=sw)

                # ---- stage B: causal attention, software-pipelined ----
                # Work units stream through (S matmul -> exp -> [mask] -> PV);
                # PV emission lags S/exp by PIPE units so the PE never waits
                # on the Act engine's exp. The 1/Z broadcast shares the PV
                # psum bank (partitions 64:128) so two PV banks fit.
                with tc.tile_pool(name="attn", bufs=8) as ap_, \
                     tc.tile_pool(name="norm", bufs=4) as np_, \
                     tc.tile_pool(name="s_psum", bufs=3, space="PSUM") as sp, \
                     tc.tile_pool(name="pv_psum", bufs=2, space="PSUM") as pvp:
                    PIPE = 2
                    pending = []  # deferred emission closures (PV matmuls, normalize)

                    def drain(n):
                        while len(pending) > n:
                            pending.pop(0)()

                    def mk_pv(pv, lhsT, rhs_p, st, sp_, c0):
                        def emit():
                            nc.tensor.matmul(pv[0:65, c0:512], lhsT=lhsT, rhs=rhs_p,
                                             start=st, stop=sp_, skip_group_check=True)
                        return emit

                    def mk_norm(pv, ct, poff, i0):
                        def emit():
                            rz = np_.tile([1, 512], F32, tag="rz")
                            with nc.allow_low_precision(reason="softmax denominators"):
                                nc.vector.reciprocal_approx_fast(out=rz, in_=pv[64:65, :])
                            bc = np_.tile([64, 512], F32, tag="bc")
                            nc.gpsimd.partition_broadcast(bc, rz)
                            nc.vector.tensor_mul(
                                out=oTn[ct][poff:poff + 64, i0:i0 + 512],
                                in0=pv[0:64, :], in1=bc,
                            )
                        return emit

                    for ch in (0, 3, 2, 1):
                        i0 = 512 * ch
                        for h in range(HPC):
                            ct, poff = h // 2, 64 * (h % 2)
                            pv = pvp.tile([128, 512], F32, tag="pv")
                            qs = q_t[ct][poff:poff + 64, i0:i0 + 512]
                            nmm = 4 * ch + 4
                            # paired full-width key blocks: (0,1), (2,3), ...
                            for m in range(2 * ch):
                                ja, jb = 2 * m, 2 * m + 1
                                s2 = sp.tile([128, 2, 512], F32, tag="s2")
                                for g, j in ((0, ja), (1, jb)):
                                    nc.tensor.matmul(
                                        s2[:, g, :],
                                        lhsT=k_t[ct][poff:poff + 64, 128 * j:128 * (j + 1)],
                                        rhs=qs, start=True, stop=True,
                                    )
                                p2 = ap_.tile([128, 2, 512], BF16, tag="p2")
                                nc.scalar.activation(out=p2, in_=s2, func=AF.Exp, scale=0.125)
                                for g, j in ((0, ja), (1, jb)):
                                    pending.append(mk_pv(pv, v_t[j][:, h, :], p2[:, g, :],
                                                         j == 0, j == nmm - 1, 0))
                                drain(PIPE)
                            # leftover full block + 3 tail blocks (w 384/256/128)
                            for j in range(4 * ch, 4 * ch + 4):
                                j0 = 128 * j
                                w = i0 + 512 - j0
                                c0 = j0 - i0
                                s2 = sp.tile([128, 2, 512], F32, tag="s2")
                                nc.tensor.matmul(
                                    s2[:, 0, 0:w],
                                    lhsT=k_t[ct][poff:poff + 64, j0:j0 + 128],
                                    rhs=q_t[ct][poff:poff + 64, j0:i0 + 512],
                                    start=True, stop=True,
                                )
                                p1 = ap_.tile([128, 512], BF16, tag="p1")
                                nc.scalar.activation(out=p1[:, 0:w], in_=s2[:, 0, 0:w],
                                                     func=AF.Exp, scale=0.125)
                                nc.vector.tensor_mul(out=p1[:, 0:128],
                                                     in0=p1[:, 0:128], in1=mask_sb)
                                pending.append(mk_pv(pv, v_t[j][:, h, :], p1[:, 0:w],
                                                     j == 0, j == nmm - 1, c0))
                                drain(PIPE)
                            # normalize lags through the pipeline like PV units
                            pending.append(mk_norm(pv, ct, poff, i0))
                    drain(0)

                # ---- stage C: output projection ----
                with tc.tile_pool(name="out_psum", bufs=6, space="PSUM") as opp:
                    dma_eng = [nc.sync, nc.scalar, nc.sync, nc.gpsimd]
                    for tch in (0, 3, 2, 1):
                        tsl = slice(512 * tch, 512 * (tch + 1))
                        for et in range(NE):
                            ps = opp.tile([128, 512], F32, tag="op")
                            for cc in range(2):
                                nc.tensor.matmul(
                                    ps,
                                    lhsT=wo_sb[cc][:, 128 * et:128 * (et + 1)],
                                    rhs=oTn[cc][:, tsl],
                                    start=(cc == 0), stop=(cc == 1),
                                )
                            if et % 2 == 0:
                                nc.scalar.copy(out=ob[et][:, tsl], in_=ps)
                            else:
                                nc.vector.tensor_copy(out=ob[et][:, tsl], in_=ps)
                            dma_eng[et % 4].dma_start(
                                out=outT[128 * et:128 * (et + 1), tsl],
                                in_=ob[et][:, tsl])

    nc.compile()
    return nc


def get_nc(repeat=1):
    key = f"nc{repeat}"
    if key not in _CACHE:
        _CACHE[key] = _build(repeat)
    return _CACHE[key]


def make_in_maps(x, w_qkv, b_qkv):
    import ml_dtypes
    cos2, sinn2, mask = _host_constants()
    x = np.asarray(x, dtype=np.float32)
    w_qkv = np.asarray(w_qkv, dtype=np.float32)
    b_qkv = np.asarray(b_qkv, dtype=np.float32)
    xT8 = [np.ascontiguousarray(x[b].T).astype(ml_dtypes.bfloat16) for b in range(B)]
    in_maps = []
    for c in range(8):
        b, hg = divmod(c, 4)
        sl = slice(CG * hg, CG * (hg + 1))
        wq = w_qkv[:, 0 * E:1 * E][:, sl]
        wk = w_qkv[:, 1 * E:2 * E][:, sl]
        wv = w_qkv[:, 2 * E:3 * E][:, sl]
        bq = b_qkv[0 * E:1 * E][sl]
        bk = b_qkv[1 * E:2 * E][sl]
        # bqk layout [128, 4]: (bq ct0, bq ct1, bk ct0, bk ct1)
        bqk = np.stack([bq[0:128], bq[128:256], bk[0:128], bk[128:256]],
                       axis=1).astype(np.float32)
        in_maps.append({
            "x8": xT8[b],
            "wqkv": np.ascontiguousarray(
                np.concatenate([wq, wk, wv], axis=1)).astype(ml_dtypes.bfloat16),
            "wo": None,  # filled by caller (needs w_out)
            "cos2": cos2, "sinn2": sinn2, "mask": mask,
            "bqk": np.ascontiguousarray(bqk),
            "ones1": np.ones((128, HPC), dtype=ml_dtypes.bfloat16),
        })
    return in_maps


def fill_wo(in_maps, w_out):
    import ml_dtypes
    w_out = np.asarray(w_out, dtype=np.float32)
    for c in range(8):
        hg = c % 4
        in_maps[c]["wo"] = np.ascontiguousarray(
            w_out[CG * hg:CG * (hg + 1), :]).astype(ml_dtypes.bfloat16)
    return in_maps


def kernel(x, w_qkv, b_qkv, w_out, b_out, _res_out=None):
    from concourse.bass_utils import run_bass_kernel_spmd

    x = np.asarray(x, dtype=np.float32)
    w_qkv = np.asarray(w_qkv, dtype=np.float32)
    b_qkv = np.asarray(b_qkv, dtype=np.float32)
    w_out = np.asarray(w_out, dtype=np.float32)
    b_out = np.asarray(b_out, dtype=np.float32)

    nc = get_nc()
    in_maps = fill_wo(make_in_maps(x, w_qkv, b_qkv), w_out)

    res = run_bass_kernel_spmd(nc, in_maps, list(range(8)))
    if _res_out is not None:
        _res_out.append(res)

    out = np.empty((B, T, E), np.float32)
    for b in range(B):
        acc = res.results[4 * b + 0]["outT"].astype(np.float32)
        for g in range(1, 4):
            acc += res.results[4 * b + g]["outT"].astype(np.float32)
        out[b] = acc.T
    bias = b_qkv[2 * E:3 * E].astype(np.float64) @ w_out.astype(np.float64) + b_out
    out += bias.astype(np.float32)[None, None, :]
    return out
